# revision 25
# baseline (speedup 1.0000x reference)
"""AttentionWithRoPE Trainium2 kernel.

Sharding: hybrid batch x head tensor-parallel over 8 cores.
Core c handles batch b = c//4 and heads [4g, 4g+4) where g = c%4
(256 of the 1024 projection features). Each core computes its heads'
q/k/v projections, RoPE, full non-causal attention, and a partial
output projection; the host sums the 4 partial outputs per batch.

Device layout notes:
- All activations are kept feature-major ([feat, seq], "transposed") so
  matmuls need no on-chip transposes anywhere.
- RoPE: rope(x) = x*cos + rot(x)*sin; rot(x) is built with partition-
  shifted (+/-32) sign-flipped DVE copies out of the projection PSUM.
- Attention uses the S^T = K_r^T(Q_r) layout; softmax denominator comes
  from a ones-column augmented V (M=65 matmul). Normalization runs
  entirely off the PE: batched DVE reciprocal + DRAM-bounce partition
  broadcast + DVE multiply.
- The A@V accumulation runs OT_LAG steps behind the S^T/exp producer
  inside the same head window; the t=1 projections are interleaved into
  the first two head windows to keep the PE dense while ACT runs exps.
"""

import sys

if "/opt/trn_rl_repo" not in sys.path:
    sys.path.insert(0, "/opt/trn_rl_repo")

import numpy as np
import ml_dtypes

B, L, DIM, H = 2, 2048, 1024, 16
HD = 64
NCORES = 8
FPC = 256          # features per core
NHC = 4            # heads per core
KCH = DIM // 128   # 8 contraction chunks of 128
BF = ml_dtypes.bfloat16

_PROG_CACHE = {}


# --------------------------------------------------------------------------
# workarounds: the walrus in this container encodes at most ONE semaphore
# wait per instruction; split extra waits onto preceding same-engine NOPs.
# --------------------------------------------------------------------------
def _install_patches():
    import concourse.tile as tile_mod
    import bass_rust as _br
    from concourse.vector_clock import ScopedClock

    if getattr(tile_mod, "_ant_wait_split_installed", False):
        return

    def _split_multi_waits(nc, ordered):
        for bb_name, insts in ordered.items():
            new_list = []
            for inst in insts:
                si = getattr(inst, "sync_info", None)
                ws = list(si.on_wait) if (si is not None and si.on_wait) else []
                if len(ws) > 1:
                    try:
                        eng = inst.engine
                        for extra in ws[:-1]:
                            nop = _br.InstNoOp(name=f"I-wsplit-{nc.next_id()}")
                            nop.engine = eng
                            nop.sync_info = _br.SyncInfo(
                                on_wait=[extra], on_update=[]
                            )
                            new_list.append(nop)
                        inst.sync_info = _br.SyncInfo(
                            on_wait=[ws[-1]], on_update=list(si.on_update or [])
                        )
                    except Exception:
                        pass
                new_list.append(inst)
            ordered[bb_name] = new_list

    _orig_lower = tile_mod.TileContext._lower_ordered_insts

    def _patched_lower(self, ordered):
        _split_multi_waits(self.nc, ordered)
        return _orig_lower(self, ordered)

    def _patched_dab(self, tick_clock, wait_clock):
        probe = self.nc.sync.nop(nofuse=True)
        wait_clock.add_sem_waits(
            probe.ins, ScopedClock({None: tick_clock.global_clock})
        )
        si = probe.ins.sync_info
        w = list(si.on_wait) if si and si.on_wait else []
        if len(w) > 1:
            probe.ins.sync_info = _br.SyncInfo(
                on_wait=w[:1], on_update=list(si.on_update or [])
            )
            for i in range(1, len(w)):
                n2 = self.nc.sync.nop(nofuse=True)
                n2.ins.sync_info = _br.SyncInfo(on_wait=[w[i]], on_update=[])
        self.nc.sync.drain()
        self.nc.all_engine_barrier()
        assert self.sems is not None
        popped = self.nc._tile_sem_poison_stack.pop()
        assert popped is self._sem_poison
        self.nc.clear_and_free_semaphores(list(self.sems.allocated().values()))
        self.nc.all_engine_barrier()

    tile_mod.TileContext._lower_ordered_insts = _patched_lower
    tile_mod.TileContext._drain_and_barrier = _patched_dab
    tile_mod._ant_wait_split_installed = True


# --------------------------------------------------------------------------
# device program
# --------------------------------------------------------------------------
def _build_program(with_bias):
    _install_patches()
    import concourse.bass as bass
    import concourse.tile as tile
    from concourse import mybir

    f32 = mybir.dt.float32
    bf16 = mybir.dt.bfloat16
    EXP = mybir.ActivationFunctionType.Exp

    nc = bass.Bass()

    xq = nc.dram_tensor("xq", [KCH, 128, L], bf16, kind="ExternalInput")
    xk = nc.dram_tensor("xk", [KCH, 128, L], bf16, kind="ExternalInput")
    xv = nc.dram_tensor("xv", [KCH, 128, L], bf16, kind="ExternalInput")
    wq = nc.dram_tensor("wq", [KCH, 128, FPC], bf16, kind="ExternalInput")
    wk = nc.dram_tensor("wk", [KCH, 128, FPC], bf16, kind="ExternalInput")
    wv = nc.dram_tensor("wv", [KCH, 128, FPC], bf16, kind="ExternalInput")
    wo = nc.dram_tensor("wo", [2, 128, DIM], bf16, kind="ExternalInput")
    if with_bias:
        bq = nc.dram_tensor("bq", [1, FPC], bf16, kind="ExternalInput")
        bk = nc.dram_tensor("bk", [1, FPC], bf16, kind="ExternalInput")
        bv = nc.dram_tensor("bv", [1, FPC], bf16, kind="ExternalInput")
    cosT = nc.dram_tensor("cosT", [2, 128, 1024], f32, kind="ExternalInput")
    sinT = nc.dram_tensor("sinT", [2, 128, 1024], f32, kind="ExternalInput")
    outT = nc.dram_tensor("outT", [KCH, 128, L], f32, kind="ExternalOutput")

    with tile.TileContext(nc) as tc:
        from contextlib import ExitStack

        with ExitStack() as ctx:
            const = ctx.enter_context(tc.tile_pool(name="const", bufs=1))
            psum = ctx.enter_context(
                tc.tile_pool(name="psum", bufs=1, space="PSUM")
            )
            inp = ctx.enter_context(tc.tile_pool(name="inp", bufs=16))
            tmpp = ctx.enter_context(tc.tile_pool(name="tmpp", bufs=4))
            ropep = ctx.enter_context(tc.tile_pool(name="ropep", bufs=1))
            vpp = ctx.enter_context(tc.tile_pool(name="vpp", bufs=16))
            ptp = ctx.enter_context(tc.tile_pool(name="ptp", bufs=10))
            oallp = ctx.enter_context(tc.tile_pool(name="oallp", bufs=1))
            onn = ctx.enter_context(tc.tile_pool(name="onn", bufs=3))
            outp = ctx.enter_context(tc.tile_pool(name="outp", bufs=2))
            dramp = ctx.enter_context(
                tc.tile_pool(name="dramp", bufs=3, space="DRAM")
            )

            def load_wchunks(name, dram):
                t = ptp.tile([128, KCH * FPC], bf16, tag="pt", name=name)
                for kc in range(KCH):
                    nc.sync.dma_start(
                        out=t[:, kc * FPC : (kc + 1) * FPC], in_=dram[kc]
                    )
                return t

            bias_sb = {}
            ones_bf = None
            if with_bias:
                for name, dram in (("bq", bq), ("bk", bk), ("bv", bv)):
                    bt = const.tile([1, FPC], bf16, name=f"{name}_sb")
                    nc.sync.dma_start(out=bt, in_=dram[:, :])
                    bias_sb[name] = bt
                ones_bf = const.tile([1, 512], bf16)
                nc.vector.memset(ones_bf, 1.0)

            ropes = {}
            for tens in ("q", "k"):
                for t_i in range(2):
                    r = ropep.tile(
                        [128, L], bf16, name=f"rope_{tens}{t_i}",
                        tag=f"rope_{tens}{t_i}",
                    )
                    ropes[tens, t_i] = r

            # ---------------- phase A unit emitters ----------------
            # One unit = projection of (tens, t_i) over both seq halves,
            # one ldweights per contraction chunk (shared by 4 matmuls),
            # then rot-copies + rope math on DVE.
            def proj_unit_steps(tens, t_i, w_sb, xch, bn):
                """Returns a list of closures; call them in order, possibly
                interleaved with other PE work."""
                pcs = [None, None]
                steps = []

                def alloc():
                    for half in range(2):
                        pcs[half] = psum.tile(
                            [128, 1024], mybir.dt.float32, tag="s", bufs=2,
                            name=f"ps_{tens}{t_i}{half}",
                        )
                steps.append(alloc)

                def mk_mm(kc):
                    def emit():
                        lo = kc * FPC + t_i * 128
                        for half in range(2):
                            hof = half * 1024
                            for qs in range(2):
                                nc.tensor.matmul(
                                    pcs[half][:, qs * 512 : (qs + 1) * 512],
                                    w_sb[:, lo : lo + 128],
                                    xch[kc][:, hof + qs * 512 : hof + (qs + 1) * 512],
                                    start=(kc == 0),
                                    stop=(kc == KCH - 1 and not with_bias),
                                )
                    return emit

                for kc in range(KCH):
                    steps.append(mk_mm(kc))

                if with_bias:
                    def bias_mm():
                        for half in range(2):
                            for qs in range(2):
                                nc.tensor.matmul(
                                    pcs[half][:, qs * 512 : (qs + 1) * 512],
                                    bias_sb[bn][:, t_i * 128 : t_i * 128 + 128],
                                    ones_bf[:, :512],
                                    start=False, stop=True,
                                )
                    steps.append(bias_mm)

                def rope_math():
                    for half in range(2):
                        ps = pcs[half]
                        hof = half * 1024
                        # rot(x): per 64-row head block swap halves, negate
                        # top; these copies run on the otherwise-idle ScalarE
                        rot = tmpp.tile([128, 1024], mybir.dt.float32,
                                        tag="tmp", name=f"rot_{tens}{t_i}{half}")
                        for blk in range(2):
                            b0 = blk * 64
                            nc.scalar.mul(
                                rot[b0 : b0 + 32, :], ps[b0 + 32 : b0 + 64, :], -1.0
                            )
                            nc.scalar.copy(
                                rot[b0 + 32 : b0 + 64, :], ps[b0 : b0 + 32, :]
                            )
                        t1 = tmpp.tile([128, 1024], mybir.dt.float32,
                                       tag="tmp", name="t1")
                        nc.vector.tensor_mul(t1, ps, cos_sb[half])
                        nc.vector.tensor_mul(rot, rot, sin_sb[half])
                        nc.vector.tensor_add(
                            ropes[tens, t_i][:, hof : hof + 1024], t1, rot
                        )
                steps.append(rope_math)
                return steps

            # ---------------- phase A + V projection ----------------
            # DMA emission order == consumption order, so the first matmul
            # starts ~5us in instead of waiting behind unrelated transfers.
            wq_sb = load_wchunks("wq_sb", wq)
            xch_q = []
            for kc in range(KCH):
                xt = inp.tile([128, L], mybir.dt.bfloat16, tag="inch",
                              name=f"x_q{kc}")
                nc.sync.dma_start(out=xt, in_=xq[kc])
                xch_q.append(xt)
            cos_sb, sin_sb = [], []
            for half in range(2):
                ct = ptp.tile([128, 1024], f32, tag="pt", name=f"cos{half}")
                nc.sync.dma_start(out=ct, in_=cosT[half])
                cos_sb.append(ct)
                st = ptp.tile([128, 1024], f32, tag="pt", name=f"sin{half}")
                nc.sync.dma_start(out=st, in_=sinT[half])
                sin_sb.append(st)
            wk_sb = load_wchunks("wk_sb", wk)
            xch_k = []
            for kc in range(KCH):
                xt = inp.tile([128, L], mybir.dt.bfloat16, tag="inch",
                              name=f"x_k{kc}")
                nc.sync.dma_start(out=xt, in_=xk[kc])
                xch_k.append(xt)

            wv_sb = load_wchunks("wv_sb", wv)
            xch_v = []
            for kc in range(KCH):
                xt = inp.tile([128, L], mybir.dt.bfloat16, tag="inch",
                              name=f"x_v{kc}")
                nc.sync.dma_start(out=xt, in_=xv[kc])
                xch_v.append(xt)

            vp_tiles = []

            def vp_unit(st):
                vps = psum.tile([128, FPC], mybir.dt.float32, tag="o", bufs=4,
                                name=f"vps{st}")
                for kc in range(KCH):
                    nc.tensor.matmul(
                        vps,
                        xch_v[kc][:, st * 128 : (st + 1) * 128],
                        wv_sb[:, kc * FPC : (kc + 1) * FPC],
                        start=(kc == 0), stop=(kc == KCH - 1 and not with_bias),
                    )
                if with_bias:
                    nc.tensor.matmul(
                        vps, ones_bf[:, :128], bias_sb["bv"],
                        start=False, stop=True,
                    )
                vt = vpp.tile([128, NHC * 65], mybir.dt.bfloat16, tag="vp",
                              name=f"vp{st}")
                vtr = vt.rearrange("p (h c) -> p h c", c=65)
                nc.vector.memset(vtr[:, :, 64], 1.0)
                for hl in range(NHC):
                    nc.vector.tensor_copy(
                        vtr[:, hl, 0:64], vps[:, hl * 64 : (hl + 1) * 64]
                    )
                vp_tiles.append(vt)

            # dense pre-attention block: vp blocks fill the rope-read PSUM
            # gaps between projection units (ordered by DMA arrival)
            for fn in proj_unit_steps("q", 0, wq_sb, xch_q, "bq"):
                fn()
            for fn in proj_unit_steps("q", 1, wq_sb, xch_q, "bq"):
                fn()
            for fn in proj_unit_steps("k", 0, wk_sb, xch_k, "bk"):
                fn()
            for st in range(4):
                vp_unit(st)
            for fn in proj_unit_steps("k", 1, wk_sb, xch_k, "bk"):
                fn()
            for st in range(4, 16):
                vp_unit(st)

            # ---------------- phase B ----------------
            oall = []
            for t_i in range(2):
                o = oallp.tile([128, L], mybir.dt.bfloat16, name=f"oall{t_i}",
                               tag=f"oall{t_i}")
                oall.append(o)

            OT_LAG = 3
            f32_ = mybir.dt.float32
            for h in range(NHC):
                t_i = h // 2
                off = (h % 2) * 64
                kr, qr = ropes["k", t_i], ropes["q", t_i]
                o_tiles = [
                    psum.tile([65, 512], f32_, tag="o", bufs=4,
                              name=f"ops_{h}_{qs}")
                    for qs in range(4)
                ]
                pts = {}
                for step in range(16 + OT_LAG):
                    kc = step
                    if kc < 16:
                        pt = ptp.tile([128, L], mybir.dt.bfloat16, tag="pt",
                                      name=f"pt_{h}_{kc}")
                        for half in range(2):
                            hof = half * 1024
                            sps = psum.tile([128, 1024], f32_, tag="s", bufs=2,
                                            name=f"sps_{h}_{kc}_{half}")
                            if half == 0:
                                # HAM warmers: the window is exp-paced and the
                                # PE would micro-idle ~0.5us/step, re-
                                # throttling the clock to 1.2 GHz. These run
                                # inside the otherwise-idle slot-wait and are
                                # overwritten by the real matmuls (start=True).
                                for _ in range(3):
                                    nc.tensor.matmul(
                                        sps[:, 0:256],
                                        kr[off : off + 64, 0:128],
                                        qr[off : off + 64, 0:256],
                                        start=True, stop=True,
                                    )
                            for qs in range(2):
                                nc.tensor.matmul(
                                    sps[:, qs * 512 : (qs + 1) * 512],
                                    kr[off : off + 64, kc * 128 : (kc + 1) * 128],
                                    qr[off : off + 64,
                                       hof + qs * 512 : hof + (qs + 1) * 512],
                                    start=True, stop=True,
                                )
                            nc.scalar.activation(
                                pt[:, hof : hof + 1024], sps, EXP, scale=0.125
                            )
                        pts[kc] = pt
                    j = step - OT_LAG
                    if 0 <= j < 16:
                        lh = vp_tiles[j][:, h * 65 : h * 65 + 65]
                        for qs in range(4):
                            nc.tensor.matmul(
                                o_tiles[qs], lh,
                                pts[j][:, qs * 512 : (qs + 1) * 512],
                                start=(j == 0), stop=(j == 15),
                            )
                        del pts[j]
                # off-PE normalization chain for head h: z path first (it is
                # the long pole: recip -> DRAM bounce -> broadcast), z rows
                # batched at partitions 0/32/64/96 so one reciprocal covers
                # all four q spans.
                zb4 = onn.tile([97, 512], f32_, tag="zb4", name=f"zb4_{h}")
                for qs in range(4):
                    nc.vector.tensor_copy(
                        zb4[qs * 32 : qs * 32 + 1, :], o_tiles[qs][64:65, :]
                    )
                zi4 = onn.tile([97, 512], f32_, tag="zi4", name=f"zi4_{h}")
                nc.vector.reciprocal(zi4, zb4)
                zd = dramp.tile([4, 512], f32_, tag="zd", name=f"zd_{h}")
                for qs in range(4):
                    nc.sync.dma_start(
                        out=zd[qs : qs + 1, :],
                        in_=zi4[qs * 32 : qs * 32 + 1, :],
                    )
                zbs, ous = [], []
                for qs in range(4):
                    zb = onn.tile([64, 512], f32_, tag="zb", name=f"zb_{h}_{qs}")
                    src = zd[qs : qs + 1, :]
                    bc = bass.AP(
                        tensor=src.tensor, offset=src.offset,
                        ap=[[0, 64]] + list(src.ap)[1:],
                    )
                    nc.gpsimd.dma_start(out=zb, in_=bc)
                    zbs.append(zb)
                    ou = onn.tile([64, 512], f32_, tag="ou", bufs=5,
                                  name=f"ou_{h}_{qs}")
                    nc.vector.tensor_copy(ou, o_tiles[qs][0:64, :])
                    ous.append(ou)
                for qs in range(4):
                    nc.vector.tensor_mul(
                        oall[t_i][off : off + 64, qs * 512 : (qs + 1) * 512],
                        ous[qs], zbs[qs],
                    )

            # ---------------- phase C: output projection ----------------
            wo_sb = []
            for t_i in range(2):
                w = const.tile([128, DIM], mybir.dt.bfloat16, name=f"wo_sb{t_i}")
                nc.sync.dma_start(out=w, in_=wo[t_i])
                wo_sb.append(w)
            # HAM warmers across the head-3 normalization latency: these
            # depend only on oall[0] (ready since window 1) so the PE keeps
            # its clock while the last norm chain drains.
            warm = psum.tile([128, 1024], f32_, tag="s", bufs=2, name="warm")
            for _ in range(24):
                nc.tensor.matmul(
                    warm[:, 0:256], wo_sb[0][:, 0:128], oall[0][:, 0:256],
                    start=True, stop=True,
                )
            for od in range(KCH):
                for half in range(2):
                    hof = half * 1024
                    cps = psum.tile([128, 1024], f32_, tag="s", bufs=2,
                                    name=f"cps_{od}_{half}")
                    for t_i in range(2):
                        for qs in range(2):
                            nc.tensor.matmul(
                                cps[:, qs * 512 : (qs + 1) * 512],
                                wo_sb[t_i][:, od * 128 : (od + 1) * 128],
                                oall[t_i][:, hof + qs * 512 : hof + (qs + 1) * 512],
                                start=(t_i == 0), stop=(t_i == 1),
                            )
                    ot = outp.tile([128, 1024], f32_, tag="ot",
                                   name=f"ot_{od}_{half}")
                    nc.vector.tensor_copy(ot, cps)
                    nc.sync.dma_start(
                        out=outT[od][:, hof : hof + 1024], in_=ot
                    )

    return nc


def _get_program(with_bias):
    key = ("nc", with_bias)
    if key not in _PROG_CACHE:
        _PROG_CACHE[key] = _build_program(with_bias)
    return _PROG_CACHE[key]


# --------------------------------------------------------------------------
# host-side helpers
# --------------------------------------------------------------------------
def _rope_tables():
    inv = (
        1.0 / (10000.0 ** (np.arange(HD // 2, dtype=np.float32) * 2.0 / HD))
    ).astype(np.float32)
    ang = np.arange(L, dtype=np.float32)[:, None] * inv[None, :]  # [L, 32]
    cosL = np.cos(ang).astype(np.float32).T  # [32, L]
    sinL = np.sin(ang).astype(np.float32).T
    blk_c = np.concatenate([cosL, cosL], axis=0)  # [64, L]
    blk_s = np.concatenate([sinL, sinL], axis=0)
    cos128 = np.ascontiguousarray(np.concatenate([blk_c, blk_c], axis=0))
    sin128 = np.ascontiguousarray(np.concatenate([blk_s, blk_s], axis=0))
    cos2 = np.ascontiguousarray(cos128.reshape(128, 2, 1024).transpose(1, 0, 2))
    sin2 = np.ascontiguousarray(sin128.reshape(128, 2, 1024).transpose(1, 0, 2))
    return cos2, sin2


def _wchunks(Mc):
    """[256, 1024] weight rows -> transposed chunked [8, 128, 256] bf16."""
    return np.ascontiguousarray(Mc.T.astype(BF)).reshape(KCH, 128, FPC)


def kernel(q, k, v, Wq, bq, Wk, bk, Wv, bv, Wo, bo, _trace=False):
    q, k, v = (np.asarray(a, dtype=np.float32) for a in (q, k, v))
    Wq, Wk, Wv, Wo = (np.asarray(a, dtype=np.float32) for a in (Wq, Wk, Wv, Wo))
    bq, bk, bv, bo = (np.asarray(a, dtype=np.float32) for a in (bq, bk, bv, bo))

    with_bias = bool(np.any(bq) or np.any(bk) or np.any(bv))
    nc = _get_program(with_bias)
    from concourse.bass_utils import run_bass_kernel_spmd

    cos_t, sin_t = _rope_tables()
    xt = {}
    for b in range(B):
        for nm, arr in (("q", q), ("k", k), ("v", v)):
            xt[nm, b] = np.ascontiguousarray(arr[b].T.astype(BF)).reshape(
                KCH, 128, L
            )

    in_maps = []
    for c in range(NCORES):
        b, g = c // 4, c % 4
        fs = slice(g * FPC, (g + 1) * FPC)
        m = {
            "xq": xt["q", b], "xk": xt["k", b], "xv": xt["v", b],
            "wq": _wchunks(Wq[fs, :]),
            "wk": _wchunks(Wk[fs, :]),
            "wv": _wchunks(Wv[fs, :]),
            "wo": np.ascontiguousarray(Wo[:, fs].T.astype(BF)).reshape(
                2, 128, DIM
            ),
            "cosT": cos_t, "sinT": sin_t,
        }
        if with_bias:
            m["bq"] = bq[fs].astype(BF).reshape(1, FPC)
            m["bk"] = bk[fs].astype(BF).reshape(1, FPC)
            m["bv"] = bv[fs].astype(BF).reshape(1, FPC)
        in_maps.append(m)

    res = run_bass_kernel_spmd(
        nc, in_maps, core_ids=list(range(NCORES)), trace=_trace
    )
    out = np.zeros((B, L, DIM), np.float32)
    for c in range(NCORES):
        b = c // 4
        oT = np.asarray(res.results[c]["outT"]).reshape(DIM, L)
        out[b] += oT.T
    out += bo[None, None, :]
    if _trace:
        return out, res
    return out


# revision 29
# speedup vs baseline: 1.2613x; 1.2613x over previous
"""AttentionWithRoPE Trainium2 kernel.

Sharding: hybrid batch x head tensor-parallel over 8 cores.
Core c handles batch b = c//4 and heads [4g, 4g+4) where g = c%4
(256 of the 1024 projection features). Each core computes its heads'
q/k/v projections, RoPE, full non-causal attention, and a partial
output projection; the host sums the 4 partial outputs per batch.

Device layout notes:
- All activations are kept feature-major ([feat, seq], "transposed") so
  matmuls need no on-chip transposes anywhere.
- RoPE: rope(x) = x*cos + rot(x)*sin; rot(x) is built with partition-
  shifted (+/-32) sign-flipped DVE copies out of the projection PSUM.
- Attention uses the S^T = K_r^T(Q_r) layout; softmax denominator comes
  from a ones-column augmented V (M=65 matmul). Normalization runs
  entirely off the PE: batched DVE reciprocal + DRAM-bounce partition
  broadcast + DVE multiply.
- The A@V accumulation runs OT_LAG steps behind the S^T/exp producer
  inside the same head window; the t=1 projections are interleaved into
  the first two head windows to keep the PE dense while ACT runs exps.
"""

import sys

if "/opt/trn_rl_repo" not in sys.path:
    sys.path.insert(0, "/opt/trn_rl_repo")

import numpy as np
import ml_dtypes

B, L, DIM, H = 2, 2048, 1024, 16
HD = 64
NCORES = 8
FPC = 256          # features per core
NHC = 4            # heads per core
KCH = DIM // 128   # 8 contraction chunks of 128
BF = ml_dtypes.bfloat16

_PROG_CACHE = {}


# --------------------------------------------------------------------------
# workarounds: the walrus in this container encodes at most ONE semaphore
# wait per instruction; split extra waits onto preceding same-engine NOPs.
# --------------------------------------------------------------------------
def _install_patches():
    import concourse.tile as tile_mod
    import bass_rust as _br
    from concourse.vector_clock import ScopedClock

    if getattr(tile_mod, "_ant_wait_split_installed", False):
        return

    def _split_multi_waits(nc, ordered):
        for bb_name, insts in ordered.items():
            new_list = []
            for inst in insts:
                si = getattr(inst, "sync_info", None)
                ws = list(si.on_wait) if (si is not None and si.on_wait) else []
                if len(ws) > 1:
                    try:
                        eng = inst.engine
                        for extra in ws[:-1]:
                            nop = _br.InstNoOp(name=f"I-wsplit-{nc.next_id()}")
                            nop.engine = eng
                            nop.sync_info = _br.SyncInfo(
                                on_wait=[extra], on_update=[]
                            )
                            new_list.append(nop)
                        inst.sync_info = _br.SyncInfo(
                            on_wait=[ws[-1]], on_update=list(si.on_update or [])
                        )
                    except Exception:
                        pass
                new_list.append(inst)
            ordered[bb_name] = new_list

    _orig_lower = tile_mod.TileContext._lower_ordered_insts

    def _patched_lower(self, ordered):
        _split_multi_waits(self.nc, ordered)
        return _orig_lower(self, ordered)

    def _patched_dab(self, tick_clock, wait_clock):
        probe = self.nc.sync.nop(nofuse=True)
        wait_clock.add_sem_waits(
            probe.ins, ScopedClock({None: tick_clock.global_clock})
        )
        si = probe.ins.sync_info
        w = list(si.on_wait) if si and si.on_wait else []
        if len(w) > 1:
            probe.ins.sync_info = _br.SyncInfo(
                on_wait=w[:1], on_update=list(si.on_update or [])
            )
            for i in range(1, len(w)):
                n2 = self.nc.sync.nop(nofuse=True)
                n2.ins.sync_info = _br.SyncInfo(on_wait=[w[i]], on_update=[])
        self.nc.sync.drain()
        self.nc.all_engine_barrier()
        assert self.sems is not None
        popped = self.nc._tile_sem_poison_stack.pop()
        assert popped is self._sem_poison
        self.nc.clear_and_free_semaphores(list(self.sems.allocated().values()))
        self.nc.all_engine_barrier()

    tile_mod.TileContext._lower_ordered_insts = _patched_lower
    tile_mod.TileContext._drain_and_barrier = _patched_dab
    tile_mod._ant_wait_split_installed = True


# --------------------------------------------------------------------------
# device program
# --------------------------------------------------------------------------
def _build_program(with_bias):
    _install_patches()
    import concourse.bass as bass
    import concourse.tile as tile
    from concourse import mybir

    f32 = mybir.dt.float32
    bf16 = mybir.dt.bfloat16
    EXP = mybir.ActivationFunctionType.Exp

    nc = bass.Bass()

    xq = nc.dram_tensor("xq", [KCH, 128, L], bf16, kind="ExternalInput")
    xk = nc.dram_tensor("xk", [KCH, 128, L], bf16, kind="ExternalInput")
    xv = nc.dram_tensor("xv", [KCH, 128, L], bf16, kind="ExternalInput")
    wq = nc.dram_tensor("wq", [KCH, 128, FPC], bf16, kind="ExternalInput")
    wk = nc.dram_tensor("wk", [KCH, 128, FPC], bf16, kind="ExternalInput")
    wv = nc.dram_tensor("wv", [KCH, 128, FPC], bf16, kind="ExternalInput")
    wo = nc.dram_tensor("wo", [2, 128, DIM], bf16, kind="ExternalInput")
    if with_bias:
        bq = nc.dram_tensor("bq", [1, FPC], bf16, kind="ExternalInput")
        bk = nc.dram_tensor("bk", [1, FPC], bf16, kind="ExternalInput")
        bv = nc.dram_tensor("bv", [1, FPC], bf16, kind="ExternalInput")
    cosT = nc.dram_tensor("cosT", [2, 128, 1024], f32, kind="ExternalInput")
    sinT = nc.dram_tensor("sinT", [2, 128, 1024], f32, kind="ExternalInput")
    outT = nc.dram_tensor("outT", [KCH, 128, L], f32, kind="ExternalOutput")

    with tile.TileContext(nc) as tc:
        from contextlib import ExitStack

        with ExitStack() as ctx:
            const = ctx.enter_context(tc.tile_pool(name="const", bufs=1))
            psum = ctx.enter_context(
                tc.tile_pool(name="psum", bufs=1, space="PSUM")
            )
            inp = ctx.enter_context(tc.tile_pool(name="inp", bufs=16))
            tmpp = ctx.enter_context(tc.tile_pool(name="tmpp", bufs=4))
            ropep = ctx.enter_context(tc.tile_pool(name="ropep", bufs=1))
            vpp = ctx.enter_context(tc.tile_pool(name="vpp", bufs=16))
            ptp = ctx.enter_context(tc.tile_pool(name="ptp", bufs=10))
            oallp = ctx.enter_context(tc.tile_pool(name="oallp", bufs=1))
            onn = ctx.enter_context(tc.tile_pool(name="onn", bufs=3))
            outp = ctx.enter_context(tc.tile_pool(name="outp", bufs=2))
            dramp = ctx.enter_context(
                tc.tile_pool(name="dramp", bufs=3, space="DRAM")
            )

            def load_wchunks(name, dram):
                t = ptp.tile([128, KCH * FPC], bf16, tag="pt", name=name)
                for kc in range(KCH):
                    nc.sync.dma_start(
                        out=t[:, kc * FPC : (kc + 1) * FPC], in_=dram[kc]
                    )
                return t

            bias_sb = {}
            ones_bf = None
            if with_bias:
                for name, dram in (("bq", bq), ("bk", bk), ("bv", bv)):
                    bt = const.tile([1, FPC], bf16, name=f"{name}_sb")
                    nc.sync.dma_start(out=bt, in_=dram[:, :])
                    bias_sb[name] = bt
                ones_bf = const.tile([1, 512], bf16)
                nc.vector.memset(ones_bf, 1.0)

            ropes = {}
            for tens in ("q", "k"):
                for t_i in range(2):
                    r = ropep.tile(
                        [128, L], bf16, name=f"rope_{tens}{t_i}",
                        tag=f"rope_{tens}{t_i}",
                    )
                    ropes[tens, t_i] = r

            # ---------------- phase A unit emitters ----------------
            # One unit = projection of (tens, t_i) over both seq halves,
            # one ldweights per contraction chunk (shared by 4 matmuls),
            # then rot-copies + rope math on DVE.
            def proj_unit_steps(tens, t_i, w_sb, xch, bn):
                """Returns a list of closures (one matmul block + one rope
                block per seq half); halves are staggered so only one PSUM
                s-slot is held at a time and units flow back-to-back."""
                steps = []

                def mk_half(half):
                    hof = half * 1024
                    box = {}

                    def mms():
                        p = psum.tile([128, 1024], mybir.dt.float32, tag="s",
                                      bufs=2, name=f"ps_{tens}{t_i}{half}")
                        box["p"] = p
                        for kc in range(KCH):
                            lo = kc * FPC + t_i * 128
                            for qs in range(2):
                                nc.tensor.matmul(
                                    p[:, qs * 512 : (qs + 1) * 512],
                                    w_sb[:, lo : lo + 128],
                                    xch[kc][:, hof + qs * 512 : hof + (qs + 1) * 512],
                                    start=(kc == 0),
                                    stop=(kc == KCH - 1 and not with_bias),
                                )
                        if with_bias:
                            for qs in range(2):
                                nc.tensor.matmul(
                                    p[:, qs * 512 : (qs + 1) * 512],
                                    bias_sb[bn][:, t_i * 128 : t_i * 128 + 128],
                                    ones_bf[:, :512],
                                    start=False, stop=True,
                                )

                    def rope_math():
                        ps = box["p"]
                        # rot(x): per 64-row head block swap halves, negate
                        # top; these copies run on the otherwise-idle ScalarE
                        rot = tmpp.tile([128, 1024], mybir.dt.float32,
                                        tag="tmp", name=f"rot_{tens}{t_i}{half}")
                        for blk in range(2):
                            b0 = blk * 64
                            nc.scalar.mul(
                                rot[b0 : b0 + 32, :], ps[b0 + 32 : b0 + 64, :], -1.0
                            )
                            nc.scalar.copy(
                                rot[b0 + 32 : b0 + 64, :], ps[b0 : b0 + 32, :]
                            )
                        t1 = tmpp.tile([128, 1024], mybir.dt.float32,
                                       tag="tmp", name="t1")
                        nc.vector.tensor_mul(t1, ps, cos_sb[half])
                        nc.vector.tensor_mul(rot, rot, sin_sb[half])
                        nc.vector.tensor_add(
                            ropes[tens, t_i][:, hof : hof + 1024], t1, rot
                        )

                    return mms, rope_math

                for half in range(2):
                    mms, rope_math = mk_half(half)
                    steps.append(mms)
                    steps.append(rope_math)
                return steps

            # ---------------- phase A + V projection ----------------
            # DMA emission order == consumption order, so the first matmul
            # starts ~5us in instead of waiting behind unrelated transfers.
            wq_sb = load_wchunks("wq_sb", wq)
            xch_q = []
            for kc in range(KCH):
                xt = inp.tile([128, L], mybir.dt.bfloat16, tag="inch",
                              name=f"x_q{kc}")
                nc.sync.dma_start(out=xt, in_=xq[kc])
                xch_q.append(xt)
            cos_sb, sin_sb = [], []
            for half in range(2):
                ct = ptp.tile([128, 1024], f32, tag="pt", name=f"cos{half}")
                nc.sync.dma_start(out=ct, in_=cosT[half])
                cos_sb.append(ct)
                st = ptp.tile([128, 1024], f32, tag="pt", name=f"sin{half}")
                nc.sync.dma_start(out=st, in_=sinT[half])
                sin_sb.append(st)
            wk_sb = load_wchunks("wk_sb", wk)
            xch_k = []
            for kc in range(KCH):
                xt = inp.tile([128, L], mybir.dt.bfloat16, tag="inch",
                              name=f"x_k{kc}")
                nc.sync.dma_start(out=xt, in_=xk[kc])
                xch_k.append(xt)

            wv_sb = load_wchunks("wv_sb", wv)
            xch_v = []
            for kc in range(KCH):
                xt = inp.tile([128, L], mybir.dt.bfloat16, tag="inch",
                              name=f"x_v{kc}")
                nc.sync.dma_start(out=xt, in_=xv[kc])
                xch_v.append(xt)

            vp_tiles = []

            def vp_unit(st):
                vps = psum.tile([128, FPC], mybir.dt.float32, tag="o", bufs=4,
                                name=f"vps{st}")
                for kc in range(KCH):
                    nc.tensor.matmul(
                        vps,
                        xch_v[kc][:, st * 128 : (st + 1) * 128],
                        wv_sb[:, kc * FPC : (kc + 1) * FPC],
                        start=(kc == 0), stop=(kc == KCH - 1 and not with_bias),
                    )
                if with_bias:
                    nc.tensor.matmul(
                        vps, ones_bf[:, :128], bias_sb["bv"],
                        start=False, stop=True,
                    )
                vt = vpp.tile([128, NHC * 65], mybir.dt.bfloat16, tag="vp",
                              name=f"vp{st}")
                vtr = vt.rearrange("p (h c) -> p h c", c=65)
                nc.vector.memset(vtr[:, :, 64], 1.0)
                for hl in range(NHC):
                    nc.vector.tensor_copy(
                        vtr[:, hl, 0:64], vps[:, hl * 64 : (hl + 1) * 64]
                    )
                vp_tiles.append(vt)

            # dense pre-attention block: vp blocks fill the rope-read PSUM
            # gaps between projection units (ordered by DMA arrival)
            for fn in proj_unit_steps("q", 0, wq_sb, xch_q, "bq"):
                fn()
            for fn in proj_unit_steps("q", 1, wq_sb, xch_q, "bq"):
                fn()
            for fn in proj_unit_steps("k", 0, wk_sb, xch_k, "bk"):
                fn()
            for st in range(4):
                vp_unit(st)
            for fn in proj_unit_steps("k", 1, wk_sb, xch_k, "bk"):
                fn()
            for st in range(4, 16):
                vp_unit(st)

            # ---------------- phase B ----------------
            oall = []
            for t_i in range(2):
                o = oallp.tile([128, L], mybir.dt.bfloat16, name=f"oall{t_i}",
                               tag=f"oall{t_i}")
                oall.append(o)

            OT_LAG = 3
            f32_ = mybir.dt.float32
            for h in range(NHC):
                t_i = h // 2
                off = (h % 2) * 64
                kr, qr = ropes["k", t_i], ropes["q", t_i]
                o_tiles = [
                    psum.tile([65, 512], f32_, tag="o", bufs=4,
                              name=f"ops_{h}_{qs}")
                    for qs in range(4)
                ]
                pts = {}
                for step in range(16 + OT_LAG):
                    kc = step
                    if kc < 16:
                        pt = ptp.tile([128, L], mybir.dt.bfloat16, tag="pt",
                                      name=f"pt_{h}_{kc}")
                        for half in range(2):
                            hof = half * 1024
                            sps = psum.tile([128, 1024], f32_, tag="s", bufs=2,
                                            name=f"sps_{h}_{kc}_{half}")
                            if half == 0:
                                # HAM warmers: the window is exp-paced and the
                                # PE would micro-idle ~0.5us/step, re-
                                # throttling the clock to 1.2 GHz. These run
                                # inside the otherwise-idle slot-wait and are
                                # overwritten by the real matmuls (start=True).
                                for _ in range(3):
                                    nc.tensor.matmul(
                                        sps[:, 0:256],
                                        kr[off : off + 64, 0:128],
                                        qr[off : off + 64, 0:256],
                                        start=True, stop=True,
                                    )
                            for qs in range(2):
                                nc.tensor.matmul(
                                    sps[:, qs * 512 : (qs + 1) * 512],
                                    kr[off : off + 64, kc * 128 : (kc + 1) * 128],
                                    qr[off : off + 64,
                                       hof + qs * 512 : hof + (qs + 1) * 512],
                                    start=True, stop=True,
                                )
                            nc.scalar.activation(
                                pt[:, hof : hof + 1024], sps, EXP, scale=0.125
                            )
                        pts[kc] = pt
                    j = step - OT_LAG
                    if 0 <= j < 16:
                        lh = vp_tiles[j][:, h * 65 : h * 65 + 65]
                        for qs in range(4):
                            nc.tensor.matmul(
                                o_tiles[qs], lh,
                                pts[j][:, qs * 512 : (qs + 1) * 512],
                                start=(j == 0), stop=(j == 15),
                            )
                        del pts[j]
                # off-PE normalization chain for head h: z path first (it is
                # the long pole: recip -> DRAM bounce -> broadcast), z rows
                # batched at partitions 0/32/64/96 so one reciprocal covers
                # all four q spans.
                zb4 = onn.tile([97, 512], f32_, tag="zb4", name=f"zb4_{h}")
                ous = []
                for qs in range(4):
                    nc.vector.tensor_copy(
                        zb4[qs * 32 : qs * 32 + 1, :], o_tiles[qs][64:65, :]
                    )
                    ou = onn.tile([64, 512], f32_, tag="ou", bufs=5,
                                  name=f"ou_{h}_{qs}")
                    nc.vector.tensor_copy(ou, o_tiles[qs][0:64, :])
                    ous.append(ou)
                zi4 = onn.tile([97, 512], f32_, tag="zi4", name=f"zi4_{h}")
                nc.vector.reciprocal(zi4, zb4)
                zd = dramp.tile([4, 512], f32_, tag="zd", name=f"zd_{h}")
                for qs in range(4):
                    nc.sync.dma_start(
                        out=zd[qs : qs + 1, :],
                        in_=zi4[qs * 32 : qs * 32 + 1, :],
                    )
                for qs in range(4):
                    zb = onn.tile([64, 512], f32_, tag="zb", name=f"zb_{h}_{qs}")
                    src = zd[qs : qs + 1, :]
                    bc = bass.AP(
                        tensor=src.tensor, offset=src.offset,
                        ap=[[0, 64]] + list(src.ap)[1:],
                    )
                    nc.gpsimd.dma_start(out=zb, in_=bc)
                    nc.vector.tensor_mul(
                        oall[t_i][off : off + 64, qs * 512 : (qs + 1) * 512],
                        ous[qs], zb,
                    )

            # ---------------- phase C: output projection ----------------
            wo_sb = []
            for t_i in range(2):
                w = const.tile([128, DIM], mybir.dt.bfloat16, name=f"wo_sb{t_i}")
                nc.sync.dma_start(out=w, in_=wo[t_i])
                wo_sb.append(w)
            # HAM warmers across the head-3 normalization latency: these
            # depend only on oall[0] (ready since window 1) so the PE keeps
            # its clock while the last norm chain drains.
            warm = psum.tile([128, 1024], f32_, tag="s", bufs=2, name="warm")
            for _ in range(80):
                nc.tensor.matmul(
                    warm[:, 0:256], wo_sb[0][:, 0:128], oall[0][:, 0:256],
                    start=True, stop=True,
                )
            for od in range(KCH):
                for half in range(2):
                    hof = half * 1024
                    cps = psum.tile([128, 1024], f32_, tag="s", bufs=2,
                                    name=f"cps_{od}_{half}")
                    for t_i in range(2):
                        for qs in range(2):
                            nc.tensor.matmul(
                                cps[:, qs * 512 : (qs + 1) * 512],
                                wo_sb[t_i][:, od * 128 : (od + 1) * 128],
                                oall[t_i][:, hof + qs * 512 : hof + (qs + 1) * 512],
                                start=(t_i == 0), stop=(t_i == 1),
                            )
                    ot = outp.tile([128, 1024], f32_, tag="ot",
                                   name=f"ot_{od}_{half}")
                    nc.vector.tensor_copy(ot, cps)
                    nc.sync.dma_start(
                        out=outT[od][:, hof : hof + 1024], in_=ot
                    )

    return nc


def _get_program(with_bias):
    key = ("nc", with_bias)
    if key not in _PROG_CACHE:
        _PROG_CACHE[key] = _build_program(with_bias)
    return _PROG_CACHE[key]


# --------------------------------------------------------------------------
# host-side helpers
# --------------------------------------------------------------------------
def _rope_tables():
    inv = (
        1.0 / (10000.0 ** (np.arange(HD // 2, dtype=np.float32) * 2.0 / HD))
    ).astype(np.float32)
    ang = np.arange(L, dtype=np.float32)[:, None] * inv[None, :]  # [L, 32]
    cosL = np.cos(ang).astype(np.float32).T  # [32, L]
    sinL = np.sin(ang).astype(np.float32).T
    blk_c = np.concatenate([cosL, cosL], axis=0)  # [64, L]
    blk_s = np.concatenate([sinL, sinL], axis=0)
    cos128 = np.ascontiguousarray(np.concatenate([blk_c, blk_c], axis=0))
    sin128 = np.ascontiguousarray(np.concatenate([blk_s, blk_s], axis=0))
    cos2 = np.ascontiguousarray(cos128.reshape(128, 2, 1024).transpose(1, 0, 2))
    sin2 = np.ascontiguousarray(sin128.reshape(128, 2, 1024).transpose(1, 0, 2))
    return cos2, sin2


def _wchunks(Mc):
    """[256, 1024] weight rows -> transposed chunked [8, 128, 256] bf16."""
    return np.ascontiguousarray(Mc.T.astype(BF)).reshape(KCH, 128, FPC)


def kernel(q, k, v, Wq, bq, Wk, bk, Wv, bv, Wo, bo, _trace=False):
    q, k, v = (np.asarray(a, dtype=np.float32) for a in (q, k, v))
    Wq, Wk, Wv, Wo = (np.asarray(a, dtype=np.float32) for a in (Wq, Wk, Wv, Wo))
    bq, bk, bv, bo = (np.asarray(a, dtype=np.float32) for a in (bq, bk, bv, bo))

    with_bias = bool(np.any(bq) or np.any(bk) or np.any(bv))
    nc = _get_program(with_bias)
    from concourse.bass_utils import run_bass_kernel_spmd

    cos_t, sin_t = _rope_tables()
    xt = {}
    for b in range(B):
        for nm, arr in (("q", q), ("k", k), ("v", v)):
            xt[nm, b] = np.ascontiguousarray(arr[b].T.astype(BF)).reshape(
                KCH, 128, L
            )

    in_maps = []
    for c in range(NCORES):
        b, g = c // 4, c % 4
        fs = slice(g * FPC, (g + 1) * FPC)
        m = {
            "xq": xt["q", b], "xk": xt["k", b], "xv": xt["v", b],
            "wq": _wchunks(Wq[fs, :]),
            "wk": _wchunks(Wk[fs, :]),
            "wv": _wchunks(Wv[fs, :]),
            "wo": np.ascontiguousarray(Wo[:, fs].T.astype(BF)).reshape(
                2, 128, DIM
            ),
            "cosT": cos_t, "sinT": sin_t,
        }
        if with_bias:
            m["bq"] = bq[fs].astype(BF).reshape(1, FPC)
            m["bk"] = bk[fs].astype(BF).reshape(1, FPC)
            m["bv"] = bv[fs].astype(BF).reshape(1, FPC)
        in_maps.append(m)

    res = run_bass_kernel_spmd(
        nc, in_maps, core_ids=list(range(NCORES)), trace=_trace
    )
    out = np.zeros((B, L, DIM), np.float32)
    for c in range(NCORES):
        b = c // 4
        oT = np.asarray(res.results[c]["outT"]).reshape(DIM, L)
        out[b] += oT.T
    out += bo[None, None, :]
    if _trace:
        return out, res
    return out


# revision 31
# speedup vs baseline: 1.2814x; 1.0159x over previous
"""AttentionWithRoPE Trainium2 kernel.

Sharding: hybrid batch x head tensor-parallel over 8 cores.
Core c handles batch b = c//4 and heads [4g, 4g+4) where g = c%4
(256 of the 1024 projection features). Each core computes its heads'
q/k/v projections, RoPE, full non-causal attention, and a partial
output projection; the host sums the 4 partial outputs per batch.

Device layout notes:
- All activations are kept feature-major ([feat, seq], "transposed") so
  matmuls need no on-chip transposes anywhere.
- RoPE: rope(x) = x*cos + rot(x)*sin; rot(x) is built with partition-
  shifted (+/-32) sign-flipped DVE copies out of the projection PSUM.
- Attention uses the S^T = K_r^T(Q_r) layout; softmax denominator comes
  from a ones-column augmented V (M=65 matmul). Normalization runs
  entirely off the PE: batched DVE reciprocal + DRAM-bounce partition
  broadcast + DVE multiply.
- The A@V accumulation runs OT_LAG steps behind the S^T/exp producer
  inside the same head window; the t=1 projections are interleaved into
  the first two head windows to keep the PE dense while ACT runs exps.
"""

import sys

if "/opt/trn_rl_repo" not in sys.path:
    sys.path.insert(0, "/opt/trn_rl_repo")

import numpy as np
import ml_dtypes

B, L, DIM, H = 2, 2048, 1024, 16
HD = 64
NCORES = 8
FPC = 256          # features per core
NHC = 4            # heads per core
KCH = DIM // 128   # 8 contraction chunks of 128
BF = ml_dtypes.bfloat16

_PROG_CACHE = {}


# --------------------------------------------------------------------------
# workarounds: the walrus in this container encodes at most ONE semaphore
# wait per instruction; split extra waits onto preceding same-engine NOPs.
# --------------------------------------------------------------------------
def _install_patches():
    import concourse.tile as tile_mod
    import bass_rust as _br
    from concourse.vector_clock import ScopedClock

    if getattr(tile_mod, "_ant_wait_split_installed", False):
        return

    def _split_multi_waits(nc, ordered):
        for bb_name, insts in ordered.items():
            new_list = []
            for inst in insts:
                si = getattr(inst, "sync_info", None)
                ws = list(si.on_wait) if (si is not None and si.on_wait) else []
                if len(ws) > 1:
                    try:
                        eng = inst.engine
                        for extra in ws[:-1]:
                            nop = _br.InstNoOp(name=f"I-wsplit-{nc.next_id()}")
                            nop.engine = eng
                            nop.sync_info = _br.SyncInfo(
                                on_wait=[extra], on_update=[]
                            )
                            new_list.append(nop)
                        inst.sync_info = _br.SyncInfo(
                            on_wait=[ws[-1]], on_update=list(si.on_update or [])
                        )
                    except Exception:
                        pass
                new_list.append(inst)
            ordered[bb_name] = new_list

    _orig_lower = tile_mod.TileContext._lower_ordered_insts

    def _patched_lower(self, ordered):
        _split_multi_waits(self.nc, ordered)
        return _orig_lower(self, ordered)

    def _patched_dab(self, tick_clock, wait_clock):
        probe = self.nc.sync.nop(nofuse=True)
        wait_clock.add_sem_waits(
            probe.ins, ScopedClock({None: tick_clock.global_clock})
        )
        si = probe.ins.sync_info
        w = list(si.on_wait) if si and si.on_wait else []
        if len(w) > 1:
            probe.ins.sync_info = _br.SyncInfo(
                on_wait=w[:1], on_update=list(si.on_update or [])
            )
            for i in range(1, len(w)):
                n2 = self.nc.sync.nop(nofuse=True)
                n2.ins.sync_info = _br.SyncInfo(on_wait=[w[i]], on_update=[])
        self.nc.sync.drain()
        self.nc.all_engine_barrier()
        assert self.sems is not None
        popped = self.nc._tile_sem_poison_stack.pop()
        assert popped is self._sem_poison
        self.nc.clear_and_free_semaphores(list(self.sems.allocated().values()))
        self.nc.all_engine_barrier()

    tile_mod.TileContext._lower_ordered_insts = _patched_lower
    tile_mod.TileContext._drain_and_barrier = _patched_dab
    tile_mod._ant_wait_split_installed = True


# --------------------------------------------------------------------------
# device program
# --------------------------------------------------------------------------
def _build_program(with_bias):
    _install_patches()
    import concourse.bass as bass
    import concourse.tile as tile
    from concourse import mybir

    f32 = mybir.dt.float32
    bf16 = mybir.dt.bfloat16
    EXP = mybir.ActivationFunctionType.Exp

    nc = bass.Bass()

    xq = nc.dram_tensor("xq", [KCH, 128, L], bf16, kind="ExternalInput")
    xk = nc.dram_tensor("xk", [KCH, 128, L], bf16, kind="ExternalInput")
    xv = nc.dram_tensor("xv", [KCH, 128, L], bf16, kind="ExternalInput")
    wq = nc.dram_tensor("wq", [KCH, 128, FPC], bf16, kind="ExternalInput")
    wk = nc.dram_tensor("wk", [KCH, 128, FPC], bf16, kind="ExternalInput")
    wv = nc.dram_tensor("wv", [KCH, 128, FPC], bf16, kind="ExternalInput")
    wo = nc.dram_tensor("wo", [2, 128, DIM], bf16, kind="ExternalInput")
    if with_bias:
        bq = nc.dram_tensor("bq", [1, FPC], bf16, kind="ExternalInput")
        bk = nc.dram_tensor("bk", [1, FPC], bf16, kind="ExternalInput")
        bv = nc.dram_tensor("bv", [1, FPC], bf16, kind="ExternalInput")
    cosT = nc.dram_tensor("cosT", [2, 128, 1024], f32, kind="ExternalInput")
    sinT = nc.dram_tensor("sinT", [2, 128, 1024], f32, kind="ExternalInput")
    outT = nc.dram_tensor("outT", [KCH, 128, L], f32, kind="ExternalOutput")

    with tile.TileContext(nc) as tc:
        from contextlib import ExitStack

        with ExitStack() as ctx:
            const = ctx.enter_context(tc.tile_pool(name="const", bufs=1))
            psum = ctx.enter_context(
                tc.tile_pool(name="psum", bufs=1, space="PSUM")
            )
            inp = ctx.enter_context(tc.tile_pool(name="inp", bufs=16))
            tmpp = ctx.enter_context(tc.tile_pool(name="tmpp", bufs=4))
            ropep = ctx.enter_context(tc.tile_pool(name="ropep", bufs=1))
            vpp = ctx.enter_context(tc.tile_pool(name="vpp", bufs=16))
            ptp = ctx.enter_context(tc.tile_pool(name="ptp", bufs=10))
            oallp = ctx.enter_context(tc.tile_pool(name="oallp", bufs=1))
            onn = ctx.enter_context(tc.tile_pool(name="onn", bufs=3))
            outp = ctx.enter_context(tc.tile_pool(name="outp", bufs=3))
            dramp = ctx.enter_context(
                tc.tile_pool(name="dramp", bufs=3, space="DRAM")
            )

            def load_wchunks(name, dram):
                t = ptp.tile([128, KCH * FPC], bf16, tag="pt", name=name)
                for kc in range(KCH):
                    nc.sync.dma_start(
                        out=t[:, kc * FPC : (kc + 1) * FPC], in_=dram[kc]
                    )
                return t

            bias_sb = {}
            ones_bf = None
            if with_bias:
                for name, dram in (("bq", bq), ("bk", bk), ("bv", bv)):
                    bt = const.tile([1, FPC], bf16, name=f"{name}_sb")
                    nc.sync.dma_start(out=bt, in_=dram[:, :])
                    bias_sb[name] = bt
                ones_bf = const.tile([1, 512], bf16)
                nc.vector.memset(ones_bf, 1.0)

            ropes = {}
            for tens in ("q", "k"):
                for t_i in range(2):
                    r = ropep.tile(
                        [128, L], bf16, name=f"rope_{tens}{t_i}",
                        tag=f"rope_{tens}{t_i}",
                    )
                    ropes[tens, t_i] = r

            # ---------------- phase A unit emitters ----------------
            # One unit = projection of (tens, t_i) over both seq halves,
            # one ldweights per contraction chunk (shared by 4 matmuls),
            # then rot-copies + rope math on DVE.
            def proj_unit_steps(tens, t_i, w_sb, xch, bn):
                """Returns a list of closures (one matmul block + one rope
                block per seq half); halves are staggered so only one PSUM
                s-slot is held at a time and units flow back-to-back."""
                steps = []

                def mk_half(half):
                    hof = half * 1024
                    box = {}

                    def mms():
                        p = psum.tile([128, 1024], mybir.dt.float32, tag="s",
                                      bufs=2, name=f"ps_{tens}{t_i}{half}")
                        box["p"] = p
                        for kc in range(KCH):
                            lo = kc * FPC + t_i * 128
                            for qs in range(2):
                                nc.tensor.matmul(
                                    p[:, qs * 512 : (qs + 1) * 512],
                                    w_sb[:, lo : lo + 128],
                                    xch[kc][:, hof + qs * 512 : hof + (qs + 1) * 512],
                                    start=(kc == 0),
                                    stop=(kc == KCH - 1 and not with_bias),
                                )
                        if with_bias:
                            for qs in range(2):
                                nc.tensor.matmul(
                                    p[:, qs * 512 : (qs + 1) * 512],
                                    bias_sb[bn][:, t_i * 128 : t_i * 128 + 128],
                                    ones_bf[:, :512],
                                    start=False, stop=True,
                                )

                    def rope_math():
                        ps = box["p"]
                        # rot(x): per 64-row head block swap halves, negate
                        # top; these copies run on the otherwise-idle ScalarE
                        rot = tmpp.tile([128, 1024], mybir.dt.float32,
                                        tag="tmp", name=f"rot_{tens}{t_i}{half}")
                        for blk in range(2):
                            b0 = blk * 64
                            nc.scalar.mul(
                                rot[b0 : b0 + 32, :], ps[b0 + 32 : b0 + 64, :], -1.0
                            )
                            nc.scalar.copy(
                                rot[b0 + 32 : b0 + 64, :], ps[b0 : b0 + 32, :]
                            )
                        t1 = tmpp.tile([128, 1024], mybir.dt.float32,
                                       tag="tmp", name="t1")
                        nc.vector.tensor_mul(t1, ps, cos_sb[half])
                        nc.vector.tensor_mul(rot, rot, sin_sb[half])
                        nc.vector.tensor_add(
                            ropes[tens, t_i][:, hof : hof + 1024], t1, rot
                        )

                    return mms, rope_math

                for half in range(2):
                    mms, rope_math = mk_half(half)
                    steps.append(mms)
                    steps.append(rope_math)
                return steps

            # ---------------- phase A + V projection ----------------
            # DMA emission order == consumption order, so the first matmul
            # starts ~5us in instead of waiting behind unrelated transfers.
            wq_sb = load_wchunks("wq_sb", wq)
            xch_q = []
            for kc in range(KCH):
                xt = inp.tile([128, L], mybir.dt.bfloat16, tag="inch",
                              name=f"x_q{kc}")
                nc.sync.dma_start(out=xt, in_=xq[kc])
                xch_q.append(xt)
            cos_sb, sin_sb = [], []
            for half in range(2):
                ct = ptp.tile([128, 1024], f32, tag="pt", name=f"cos{half}")
                nc.sync.dma_start(out=ct, in_=cosT[half])
                cos_sb.append(ct)
                st = ptp.tile([128, 1024], f32, tag="pt", name=f"sin{half}")
                nc.sync.dma_start(out=st, in_=sinT[half])
                sin_sb.append(st)
            wk_sb = load_wchunks("wk_sb", wk)
            xch_k = []
            for kc in range(KCH):
                xt = inp.tile([128, L], mybir.dt.bfloat16, tag="inch",
                              name=f"x_k{kc}")
                nc.sync.dma_start(out=xt, in_=xk[kc])
                xch_k.append(xt)

            wv_sb = load_wchunks("wv_sb", wv)
            wo_sb = []
            for t_i in range(2):
                w = const.tile([128, DIM], mybir.dt.bfloat16, name=f"wo_sb{t_i}")
                nc.sync.dma_start(out=w, in_=wo[t_i])
                wo_sb.append(w)
            xch_v = []
            for kc in range(KCH):
                xt = inp.tile([128, L], mybir.dt.bfloat16, tag="inch",
                              name=f"x_v{kc}")
                nc.sync.dma_start(out=xt, in_=xv[kc])
                xch_v.append(xt)

            vp_tiles = []

            def vp_unit(st):
                vps = psum.tile([128, FPC], mybir.dt.float32, tag="o", bufs=4,
                                name=f"vps{st}")
                for kc in range(KCH):
                    nc.tensor.matmul(
                        vps,
                        xch_v[kc][:, st * 128 : (st + 1) * 128],
                        wv_sb[:, kc * FPC : (kc + 1) * FPC],
                        start=(kc == 0), stop=(kc == KCH - 1 and not with_bias),
                    )
                if with_bias:
                    nc.tensor.matmul(
                        vps, ones_bf[:, :128], bias_sb["bv"],
                        start=False, stop=True,
                    )
                vt = vpp.tile([128, NHC * 65], mybir.dt.bfloat16, tag="vp",
                              name=f"vp{st}")
                vtr = vt.rearrange("p (h c) -> p h c", c=65)
                nc.vector.memset(vtr[:, :, 64], 1.0)
                for hl in range(NHC):
                    nc.vector.tensor_copy(
                        vtr[:, hl, 0:64], vps[:, hl * 64 : (hl + 1) * 64]
                    )
                vp_tiles.append(vt)

            # dense pre-attention block: vp blocks fill the rope-read PSUM
            # gaps between projection units (ordered by DMA arrival)
            for fn in proj_unit_steps("q", 0, wq_sb, xch_q, "bq"):
                fn()
            for fn in proj_unit_steps("q", 1, wq_sb, xch_q, "bq"):
                fn()
            for fn in proj_unit_steps("k", 0, wk_sb, xch_k, "bk"):
                fn()
            for st in range(4):
                vp_unit(st)
            for fn in proj_unit_steps("k", 1, wk_sb, xch_k, "bk"):
                fn()
            for st in range(4, 13):
                vp_unit(st)

            # ---------------- phase B ----------------
            oall = []
            for t_i in range(2):
                o = oallp.tile([128, L], mybir.dt.bfloat16, name=f"oall{t_i}",
                               tag=f"oall{t_i}")
                oall.append(o)

            OT_LAG = 3
            f32_ = mybir.dt.float32
            for h in range(NHC):
                t_i = h // 2
                off = (h % 2) * 64
                kr, qr = ropes["k", t_i], ropes["q", t_i]
                o_tiles = None
                pts = {}
                for step in range(16 + OT_LAG):
                    kc = step
                    if h == 0 and step < 3:
                        vp_unit(13 + step)
                    if kc < 16:
                        pt = ptp.tile([128, L], mybir.dt.bfloat16, tag="pt",
                                      name=f"pt_{h}_{kc}")
                        for half in range(2):
                            hof = half * 1024
                            sps = psum.tile([128, 1024], f32_, tag="s", bufs=2,
                                            name=f"sps_{h}_{kc}_{half}")
                            if half == 0:
                                # HAM warmers: the window is exp-paced and the
                                # PE would micro-idle ~0.5us/step, re-
                                # throttling the clock to 1.2 GHz. These run
                                # inside the otherwise-idle slot-wait and are
                                # overwritten by the real matmuls (start=True).
                                for _ in range(3):
                                    nc.tensor.matmul(
                                        sps[:, 0:256],
                                        kr[off : off + 64, 0:128],
                                        qr[off : off + 64, 0:256],
                                        start=True, stop=True,
                                    )
                            for qs in range(2):
                                nc.tensor.matmul(
                                    sps[:, qs * 512 : (qs + 1) * 512],
                                    kr[off : off + 64, kc * 128 : (kc + 1) * 128],
                                    qr[off : off + 64,
                                       hof + qs * 512 : hof + (qs + 1) * 512],
                                    start=True, stop=True,
                                )
                            nc.scalar.activation(
                                pt[:, hof : hof + 1024], sps, EXP, scale=0.125
                            )
                        pts[kc] = pt
                    j = step - OT_LAG
                    if 0 <= j < 16:
                        if o_tiles is None:
                            o_tiles = [
                                psum.tile([65, 512], f32_, tag="o", bufs=4,
                                          name=f"ops_{h}_{qs}")
                                for qs in range(4)
                            ]
                        lh = vp_tiles[j][:, h * 65 : h * 65 + 65]
                        for qs in range(4):
                            nc.tensor.matmul(
                                o_tiles[qs], lh,
                                pts[j][:, qs * 512 : (qs + 1) * 512],
                                start=(j == 0), stop=(j == 15),
                            )
                        del pts[j]
                # off-PE normalization chain for head h: z path first (it is
                # the long pole: recip -> DRAM bounce -> broadcast), z rows
                # batched at partitions 0/32/64/96 so one reciprocal covers
                # all four q spans.
                zb4 = onn.tile([97, 512], f32_, tag="zb4", name=f"zb4_{h}")
                ous = []
                for qs in range(4):
                    nc.vector.tensor_copy(
                        zb4[qs * 32 : qs * 32 + 1, :], o_tiles[qs][64:65, :]
                    )
                    ou = onn.tile([64, 512], f32_, tag="ou", bufs=5,
                                  name=f"ou_{h}_{qs}")
                    nc.vector.tensor_copy(ou, o_tiles[qs][0:64, :])
                    ous.append(ou)
                zi4 = onn.tile([97, 512], f32_, tag="zi4", name=f"zi4_{h}")
                nc.vector.reciprocal(zi4, zb4)
                zd = dramp.tile([4, 512], f32_, tag="zd", name=f"zd_{h}")
                for qs in range(4):
                    nc.sync.dma_start(
                        out=zd[qs : qs + 1, :],
                        in_=zi4[qs * 32 : qs * 32 + 1, :],
                    )
                for qs in range(4):
                    zb = onn.tile([64, 512], f32_, tag="zb", name=f"zb_{h}_{qs}")
                    src = zd[qs : qs + 1, :]
                    bc = bass.AP(
                        tensor=src.tensor, offset=src.offset,
                        ap=[[0, 64]] + list(src.ap)[1:],
                    )
                    nc.gpsimd.dma_start(out=zb, in_=bc)
                    nc.vector.tensor_mul(
                        oall[t_i][off : off + 64, qs * 512 : (qs + 1) * 512],
                        ous[qs], zb,
                    )

            # ---------------- phase C: output projection ----------------
            # HAM warmers across the head-3 normalization latency: these
            # depend only on oall[0] (ready since window 1) so the PE keeps
            # its clock while the last norm chain drains.
            warm = psum.tile([128, 1024], f32_, tag="s", bufs=2, name="warm")
            for _ in range(80):
                nc.tensor.matmul(
                    warm[:, 0:256], wo_sb[0][:, 0:128], oall[0][:, 0:256],
                    start=True, stop=True,
                )
            for od in range(KCH):
                for half in range(2):
                    hof = half * 1024
                    cps = psum.tile([128, 1024], f32_, tag="s", bufs=2,
                                    name=f"cps_{od}_{half}")
                    for t_i in range(2):
                        for qs in range(2):
                            nc.tensor.matmul(
                                cps[:, qs * 512 : (qs + 1) * 512],
                                wo_sb[t_i][:, od * 128 : (od + 1) * 128],
                                oall[t_i][:, hof + qs * 512 : hof + (qs + 1) * 512],
                                start=(t_i == 0), stop=(t_i == 1),
                            )
                    ot = outp.tile([128, 1024], f32_, tag="ot", bufs=3,
                                   name=f"ot_{od}_{half}")
                    if (od * 2 + half) % 2 == 0:
                        nc.vector.tensor_copy(ot, cps)
                    else:
                        nc.scalar.copy(ot, cps)
                    nc.sync.dma_start(
                        out=outT[od][:, hof : hof + 1024], in_=ot
                    )

    return nc


def _get_program(with_bias):
    key = ("nc", with_bias)
    if key not in _PROG_CACHE:
        _PROG_CACHE[key] = _build_program(with_bias)
    return _PROG_CACHE[key]


# --------------------------------------------------------------------------
# host-side helpers
# --------------------------------------------------------------------------
def _rope_tables():
    inv = (
        1.0 / (10000.0 ** (np.arange(HD // 2, dtype=np.float32) * 2.0 / HD))
    ).astype(np.float32)
    ang = np.arange(L, dtype=np.float32)[:, None] * inv[None, :]  # [L, 32]
    cosL = np.cos(ang).astype(np.float32).T  # [32, L]
    sinL = np.sin(ang).astype(np.float32).T
    blk_c = np.concatenate([cosL, cosL], axis=0)  # [64, L]
    blk_s = np.concatenate([sinL, sinL], axis=0)
    cos128 = np.ascontiguousarray(np.concatenate([blk_c, blk_c], axis=0))
    sin128 = np.ascontiguousarray(np.concatenate([blk_s, blk_s], axis=0))
    cos2 = np.ascontiguousarray(cos128.reshape(128, 2, 1024).transpose(1, 0, 2))
    sin2 = np.ascontiguousarray(sin128.reshape(128, 2, 1024).transpose(1, 0, 2))
    return cos2, sin2


def _wchunks(Mc):
    """[256, 1024] weight rows -> transposed chunked [8, 128, 256] bf16."""
    return np.ascontiguousarray(Mc.T.astype(BF)).reshape(KCH, 128, FPC)


def kernel(q, k, v, Wq, bq, Wk, bk, Wv, bv, Wo, bo, _trace=False):
    q, k, v = (np.asarray(a, dtype=np.float32) for a in (q, k, v))
    Wq, Wk, Wv, Wo = (np.asarray(a, dtype=np.float32) for a in (Wq, Wk, Wv, Wo))
    bq, bk, bv, bo = (np.asarray(a, dtype=np.float32) for a in (bq, bk, bv, bo))

    with_bias = bool(np.any(bq) or np.any(bk) or np.any(bv))
    nc = _get_program(with_bias)
    from concourse.bass_utils import run_bass_kernel_spmd

    cos_t, sin_t = _rope_tables()
    xt = {}
    for b in range(B):
        for nm, arr in (("q", q), ("k", k), ("v", v)):
            xt[nm, b] = np.ascontiguousarray(arr[b].T.astype(BF)).reshape(
                KCH, 128, L
            )

    in_maps = []
    for c in range(NCORES):
        b, g = c // 4, c % 4
        fs = slice(g * FPC, (g + 1) * FPC)
        m = {
            "xq": xt["q", b], "xk": xt["k", b], "xv": xt["v", b],
            "wq": _wchunks(Wq[fs, :]),
            "wk": _wchunks(Wk[fs, :]),
            "wv": _wchunks(Wv[fs, :]),
            "wo": np.ascontiguousarray(Wo[:, fs].T.astype(BF)).reshape(
                2, 128, DIM
            ),
            "cosT": cos_t, "sinT": sin_t,
        }
        if with_bias:
            m["bq"] = bq[fs].astype(BF).reshape(1, FPC)
            m["bk"] = bk[fs].astype(BF).reshape(1, FPC)
            m["bv"] = bv[fs].astype(BF).reshape(1, FPC)
        in_maps.append(m)

    res = run_bass_kernel_spmd(
        nc, in_maps, core_ids=list(range(NCORES)), trace=_trace
    )
    out = np.zeros((B, L, DIM), np.float32)
    for c in range(NCORES):
        b = c // 4
        oT = np.asarray(res.results[c]["outT"]).reshape(DIM, L)
        out[b] += oT.T
    out += bo[None, None, :]
    if _trace:
        return out, res
    return out
